# revision 22
# baseline (speedup 1.0000x reference)
"""Trainium2 Bass kernel for nn_Actor (diagonal complex LRU, last-step output).

Math: the reference scans x_t = lam*x_{t-1} + (gamma*B) u_t over L=2048 steps
and keeps y[:, -1, :].  The last state collapses to
    x_L[n] = sum_k lam[n]^k * (Bhat @ u_{L-1-k})[n]
Because |lam| <= 0.99 by construction (LRU stable init), the sum truncates:
per mode only K_n = ln(eps)/ln|lam_n| terms matter.  We sort modes by |lam|
(host-side permutation of the diagonal; the output is permutation invariant),
give the top 128 modes KT0 (=4) time-tiles of 128 steps and the bottom 128
modes 1 tile.  Per-core device work:
    v[n, b, h] = sum_k W[k, n] * u[b, L-1-k, h]     (TensorE, PSUM accum;
                 tile 0 bf16, tiles 1-2 fp8 DoubleRow pair, tile 3 fp8)
    q1..q4     = B-products of v                    (VectorE bf16, + GpSimd)
    ypsum[o,b,h] = +-C-projection of q1..q4         (TensorE, contracts n;
                 linearity folds the complex combines into the lhsT sign)
    y[o, b]    = sum_h ypsum + D u_last             (VectorE reduce + add)
W = lam^k tables, scaled B, +-C^T, D^T are all precomputed on host (float64)
and shipped bf16/fp8; u is host-repacked time-reversed/transposed so every
DMA is a handful of large contiguous transfers.  Dummy warm-up matmuls keep
the PE ramped to full clock while the DMAs land.

Sharding: data-parallel over batch (64 -> 8 per core) on 8 NeuronCores,
no collectives; host concatenates per-core outputs.
"""

import sys

sys.path.insert(0, "/opt/trn_rl_repo")

import numpy as np
from ml_dtypes import bfloat16 as np_bf16
from ml_dtypes import float8_e4m3 as np_f8

import concourse.bass as bass
import concourse.tile as tile
from concourse import bacc, mybir
from concourse.bass_utils import run_bass_kernel_spmd

B, L, H, O, N = 64, 2048, 128, 128, 256
NCORES = 8
BS = B // NCORES  # 8 batches per core
EPS_TAIL = 1.2e-2  # per-mode truncation tail; exact agg err ~2e-4 (bf16 noise dominates)
F32 = mybir.dt.float32
BF16 = mybir.dt.bfloat16
F8 = mybir.dt.float8e4
N_WARM = 34  # PE ramp-up dummies while DMAs land (small: free dim 128)


def build(kt0=4, kt1=1, fp8=True):
    nc = bacc.Bacc("TRN2", target_bir_lowering=False, debug=False)
    assert kt1 == 1, "bottom-half modes always fit one tile for this init"
    # fp8 path: DoubleRow pair (j1,j2) for kt0>=3, plus single j3 for kt0==4
    fp8 = fp8 and kt0 in (3, 4)

    # u0w: W slot0 (re_h0, im_h0, re_h1, im_h1) + u tile0 (critical first DMA)
    u0w_d = nc.dram_tensor("u0w", [128, 12 * 128], BF16, kind="ExternalInput")
    if fp8:
        # b12: per kt in {j1,j2}: [w_re | w_im | u(8)] -> [128, 2, 10, 128]
        b12_d = nc.dram_tensor("b12", [128, 20 * 128], F8, kind="ExternalInput")
        if kt0 == 4:
            b3_d = nc.dram_tensor("b3", [128, 10 * 128], F8, kind="ExternalInput")
        n_extra = 0
    else:
        n_extra = max(kt0 - 1, 1)
        bx_d = nc.dram_tensor(
            "bx", [128, n_extra * 10 * 128], BF16, kind="ExternalInput"
        )
    # params: bhre0 bhre1 bhim0 bhim1 cret0 cret1 ncret0 ncret1 ncimt0 ncimt1 dT (11*128) + ulT(8)
    p_d = nc.dram_tensor("P", [128, 11 * 128 + BS], BF16, kind="ExternalInput")
    out_d = nc.dram_tensor("out", [O, BS], F32, kind="ExternalOutput")

    mult = mybir.AluOpType.mult
    add = mybir.AluOpType.add
    DR = mybir.MatmulPerfMode.DoubleRow

    with tile.TileContext(nc) as tc:
        with (
            tc.tile_pool(name="cp", bufs=1) as cp,
            tc.tile_pool(name="psum", bufs=1, space=bass.MemorySpace.PSUM) as pp,
        ):
            # warm tile memset first: it unblocks the PE ramp-up dummies
            warm = cp.tile([128, 128], BF16, tag="warm")
            nc.gpsimd.memset(warm[:], 0.125)

            # ---- DMAs (sync: u/W stream; gpsimd: params; sync: out) -----
            u0w = cp.tile([128, 12, 128], BF16, tag="u0w")
            nc.sync.dma_start(u0w[:], u0w_d.reshape([128, 12, 128])[:])
            if fp8:
                b12 = cp.tile([128, 2, 10, 128], F8, tag="b12")
                nc.sync.dma_start(b12[:], b12_d.reshape([128, 2, 10, 128])[:])
                if kt0 == 4:
                    b3 = cp.tile([128, 10, 128], F8, tag="b3")
                    nc.sync.dma_start(b3[:], b3_d.reshape([128, 10, 128])[:])
            else:
                bx = cp.tile([128, n_extra, 10, 128], BF16, tag="bx")
                if kt0 > 1:
                    nc.sync.dma_start(
                        bx[:], bx_d.reshape([128, n_extra, 10, 128])[:]
                    )
            # params go via gpsimd's SWDGE queue so they don't delay the
            # fp8 tile blobs on the shared DMA bus (sync) or the scalar queue
            p_sb = cp.tile([128, 11 * 128 + BS], BF16, tag="p_sb")
            nc.gpsimd.dma_start(p_sb[:], p_d[:])

            w0 = {  # (component, half) -> lhsT
                (0, 0): u0w[:, 0, :], (1, 0): u0w[:, 1, :],
                (0, 1): u0w[:, 2, :], (1, 1): u0w[:, 3, :],
            }

            def u0sl(bh):  # u tile0, batch half
                return u0w[:, 4 + bh * 4 : 8 + bh * 4, :]
            bhre = [p_sb[:, hf * 128 : (hf + 1) * 128] for hf in range(2)]
            bhim = [p_sb[:, (2 + hf) * 128 : (3 + hf) * 128] for hf in range(2)]
            cret = [p_sb[:, (4 + hf) * 128 : (5 + hf) * 128] for hf in range(2)]
            ncret = [p_sb[:, (6 + hf) * 128 : (7 + hf) * 128] for hf in range(2)]
            ncimt = [p_sb[:, (8 + hf) * 128 : (9 + hf) * 128] for hf in range(2)]
            dT = p_sb[:, 10 * 128 : 11 * 128]
            ulT = p_sb[:, 11 * 128 : 11 * 128 + BS]

            # ---- PSUM ---------------------------------------------------
            pv0re = pp.tile([128, BS, H], F32, tag="pv0re")
            pv0im = pp.tile([128, BS, H], F32, tag="pv0im")
            pv1re = pp.tile([128, BS, H], F32, tag="pv1re")
            pv1im = pp.tile([128, BS, H], F32, tag="pv1im")

            # ---- PE warm-up: keep the clock ramped while DMAs land ------
            for _ in range(N_WARM):
                nc.tensor.matmul(
                    pv0re[:, 0:1, :], warm[:], warm[:], start=True, stop=True
                )

            # ---- main contraction over time tiles -----------------------
            # order: h1 first (its epilogue starts earliest), then h0 tiles
            def bsl(ap, bh):
                return ap[:, bh * 4 : (bh + 1) * 4, :]

            for bh in range(2):
                for c, pv in ((0, pv1re), (1, pv1im)):  # c: 0=re 1=im
                    nc.tensor.matmul(
                        bsl(pv, bh), w0[(c, 1)], u0sl(bh),
                        start=True, stop=True,
                    )
            for bh in range(2):
                for c, pv in ((0, pv0re), (1, pv0im)):
                    nc.tensor.matmul(
                        bsl(pv, bh), w0[(c, 0)], u0sl(bh),
                        start=True, stop=(kt0 == 1),
                    )
            if fp8:
                for bh in range(2):
                    for c, pv in ((0, pv0re), (1, pv0im)):
                        nc.tensor.matmul(
                            bsl(pv, bh),
                            b12[:, :, c, :],
                            b12[:, :, 2 + bh * 4 : 2 + (bh + 1) * 4, :],
                            start=False, stop=(kt0 == 3), perf_mode=DR,
                        )
                        if kt0 == 4:
                            nc.tensor.matmul(
                                bsl(pv, bh), b3[:, c, :],
                                b3[:, 2 + bh * 4 : 2 + (bh + 1) * 4, :],
                                start=False, stop=True,
                            )
            else:
                for j in range(1, kt0):
                    for bh in range(2):
                        for c, pv in ((0, pv0re), (1, pv0im)):
                            nc.tensor.matmul(
                                bsl(pv, bh), bx[:, j - 1, c, :],
                                bx[:, j - 1, 2 + bh * 4 : 2 + (bh + 1) * 4, :],
                                start=False, stop=(j == kt0 - 1),
                            )

            # ---- epilogue ----------------------------------------------
            # scalar: PSUM->SBUF bf16 copies (b-split, ordered by readiness)
            # vector: full-width B-products; tensor: +-C projection
            ypsum = pp.tile([128, BS, H], F32, tag="pv1re", name="ypsum")
            p2 = pp.tile([O, BS], F32, tag="pv1im", name="p2")

            def sv_copy(name, pv):
                t = cp.tile([128, BS, H], BF16, tag=name, name=name)
                for bh in range(2):
                    nc.scalar.copy(bsl(t, bh), bsl(pv, bh))
                return t

            sv_re_1 = sv_copy("sv_re_1", pv1re)
            sv_im_1 = sv_copy("sv_im_1", pv1im)
            sv_re_0 = sv_copy("sv_re_0", pv0re)
            sv_im_0 = sv_copy("sv_im_0", pv0im)

            ystate = {0: None, 1: None}  # bank -> started?

            def qproj(name, sv, bb_ap, lhs, hf, stop=False):
                bb = bb_ap[:, None, :].broadcast_to([128, BS, H])
                q = cp.tile([128, BS, H], BF16, tag=f"q{name}", name=f"q{name}")
                nc.vector.tensor_tensor(q[:], sv[:], bb, mult)
                for bh in range(2):
                    nc.tensor.matmul(
                        bsl(ypsum, bh), lhs, bsl(q, bh),
                        start=(ystate[bh] is None), stop=stop,
                    )
                    ystate[bh] = True

            # order follows copy availability: re-based products first
            qproj("1a", sv_re_1, bhre[1], cret[1], 1)
            qproj("1d", sv_re_1, bhim[1], ncimt[1], 1)
            qproj("1b", sv_im_1, bhim[1], ncret[1], 1)
            qproj("1c", sv_im_1, bhre[1], ncimt[1], 1)
            nc.tensor.matmul(p2[:], dT, ulT, start=True, stop=True)
            qproj("0a", sv_re_0, bhre[0], cret[0], 0)
            qproj("0d", sv_re_0, bhim[0], ncimt[0], 0)
            qproj("0b", sv_im_0, bhim[0], ncret[0], 0)
            qproj("0c", sv_im_0, bhre[0], ncimt[0], 0, stop=True)

            # ---- final reduce over h + D add + store --------------------
            out_sb = cp.tile([O, BS], F32, tag="out_sb")
            ysum = cp.tile([O, BS], F32, tag="ysum")
            nc.vector.tensor_reduce(ysum[:], ypsum[:], mybir.AxisListType.X, add)
            nc.vector.tensor_tensor(out_sb[:], ysum[:], p2[:], add)
            nc.sync.dma_start(out_d[:, :], out_sb[:])

    nc.compile()
    return nc


_NC_CACHE = {}


def _get_nc(kt0=4, kt1=1):
    key = (kt0, kt1)
    if key not in _NC_CACHE:
        _NC_CACHE[key] = build(kt0, kt1)
    return _NC_CACHE[key]


def _plan(inputs):
    """Host-side: mode sort, tile counts, lam-power tables, param packing."""
    nu = np.asarray(inputs["nu_log"], np.float64)
    th = np.asarray(inputs["theta_log"], np.float64)
    gm = np.asarray(inputs["gamma_log"], np.float64)
    lam_abs = np.exp(-np.exp(nu))
    order = np.argsort(-lam_abs)  # descending |lam|
    sl = lam_abs[order]
    K = np.ceil(np.log(EPS_TAIL) / np.log(np.minimum(sl, 1.0 - 1e-12)))
    K = np.clip(K, 1, L).astype(int)
    kt0 = min(max(1, int(np.ceil(K[:128].max() / 128))), L // 128)
    kt1 = max(1, int(np.ceil(K[128:].max() / 128)))
    if kt1 > 1:  # can't happen for the LRU init; degrade gracefully
        kt0 = max(kt0, kt1)
        kt1 = 1
        order = np.arange(N)  # no sort needed if everything runs long

    lam = np.exp(-np.exp(nu[order]) + 1j * np.exp(th[order]))
    ks = np.arange(128, dtype=np.float64)[:, None]

    def wslot(j, hf):  # [128p, 128n] complex
        base = lam[hf * 128 : (hf + 1) * 128]
        return base ** (128.0 * j + ks)

    Bre = np.asarray(inputs["B_re"], np.float64)[order]
    Bim = np.asarray(inputs["B_im"], np.float64)[order]
    g = np.exp(gm[order])[:, None]
    bhre, bhim = Bre * g, Bim * g
    Cre = np.asarray(inputs["C_re"], np.float64)[:, order]
    Cim = np.asarray(inputs["C_im"], np.float64)[:, order]
    D = np.asarray(inputs["D"], np.float64)

    P = np.zeros((128, 11 * 128 + BS), np.float64)
    for hf in range(2):
        s = hf * 128
        P[:, hf * 128 : (hf + 1) * 128] = bhre[s : s + 128]
        P[:, (2 + hf) * 128 : (3 + hf) * 128] = bhim[s : s + 128]
        P[:, (4 + hf) * 128 : (5 + hf) * 128] = Cre[:, s : s + 128].T
        P[:, (6 + hf) * 128 : (7 + hf) * 128] = -Cre[:, s : s + 128].T
        P[:, (8 + hf) * 128 : (9 + hf) * 128] = -Cim[:, s : s + 128].T
    P[:, 10 * 128 : 11 * 128] = D.T
    return {"kt0": kt0, "kt1": kt1, "wslot": wslot, "P": P}


def _make_in_maps(inputs, plan=None):
    if plan is None:
        plan = _plan(inputs)
    kt0 = plan["kt0"]
    fp8 = kt0 in (3, 4)
    wslot = plan["wslot"]
    u = np.asarray(inputs["dynamics_disturbance_time_window"], np.float32)
    urev = np.ascontiguousarray(u[:, ::-1, :][:, : kt0 * 128, :]).reshape(
        B, kt0, 128, H
    )

    # u0w: [128p, 12, 128]: W slot0 re/im both halves + u tile0
    w0 = np.zeros((128, 4, 128), np.float64)
    w00, w01 = wslot(0, 0), wslot(0, 1)
    w0[:, 0], w0[:, 1] = w00.real, w00.imag
    w0[:, 2], w0[:, 3] = w01.real, w01.imag

    if fp8:
        wb = np.zeros((128, 2, 2, 128), np.float64)  # [p, ktpair, c, n]
        for i, j in enumerate((1, 2)):
            wj = wslot(j, 0)
            wb[:, i, 0], wb[:, i, 1] = wj.real, wj.imag
        w3 = wslot(3, 0) if kt0 == 4 else None
    else:
        n_extra = max(kt0 - 1, 1)
        bx = np.zeros((128, n_extra, 10, 128), np.float64)
        for j in range(1, kt0):
            wj = wslot(j, 0)
            bx[:, j - 1, 0], bx[:, j - 1, 1] = wj.real, wj.imag

    in_maps = []
    for c in range(NCORES):
        ub = urev[c * BS : (c + 1) * BS]  # [BS, kt0, 128, H]
        u0w = np.zeros((128, 12, 128), np.float32)
        u0w[:, 0:4, :] = w0
        u0w[:, 4:12, :] = ub[:, 0].transpose(1, 0, 2)
        m = {"u0w": u0w.astype(np_bf16).reshape(128, 12 * 128)}
        if fp8:
            b12 = np.zeros((128, 2, 10, 128), np.float32)
            b12[:, :, 0:2, :] = wb
            for i, j in enumerate((1, 2)):
                b12[:, i, 2:10, :] = ub[:, j].transpose(1, 0, 2)
            m["b12"] = b12.astype(np_f8).reshape(128, 20 * 128)
            if kt0 == 4:
                b3 = np.zeros((128, 10, 128), np.float32)
                b3[:, 0], b3[:, 1] = w3.real, w3.imag
                b3[:, 2:10, :] = ub[:, 3].transpose(1, 0, 2)
                m["b3"] = b3.astype(np_f8).reshape(128, 10 * 128)
        else:
            bxc = bx.astype(np.float32).copy()
            for j in range(1, kt0):
                bxc[:, j - 1, 2:10, :] = ub[:, j].transpose(1, 0, 2)
            m["bx"] = bxc.astype(np_bf16).reshape(128, n_extra * 10 * 128)
        P = plan["P"].copy()
        P[:, 11 * 128 : 11 * 128 + BS] = (
            u[c * BS : (c + 1) * BS, L - 1, :].astype(np.float64).T
        )
        m["P"] = P.astype(np_bf16)
        in_maps.append(m)
    return in_maps


def _ensure_profile_hook():
    """The agent image's antenv lacks axon_hooks; shim it and register the
    ctypes NTFF hook so run_bass_kernel_spmd(trace=True) can profile."""
    import types

    if "antenv.axon_hooks" in sys.modules:
        return
    mod = types.ModuleType("antenv.axon_hooks")
    mod._hook = None
    mod.set_axon_ntff_profile_hook = lambda h: setattr(mod, "_hook", h)
    mod.get_axon_ntff_profile_hook = lambda: mod._hook
    sys.modules["antenv.axon_hooks"] = mod
    try:
        from trn_agent_boot.trn_boot import _ntff_profile_via_ctypes

        mod._hook = _ntff_profile_via_ctypes("/opt/axon/libaxon_pjrt.so")
    except Exception as e:
        print(f"profile hook setup failed: {e}", file=sys.stderr)


def run(inputs, trace=False, tmpdir=None):
    if trace:
        _ensure_profile_hook()
    plan = _plan(inputs)
    nc = _get_nc(plan["kt0"], plan["kt1"])
    in_maps = _make_in_maps(inputs, plan)
    res = run_bass_kernel_spmd(
        nc, in_maps, list(range(NCORES)), trace=trace, tmpdir=tmpdir
    )
    out = np.concatenate(
        [np.asarray(res.results[i]["out"]).T for i in range(NCORES)], axis=0
    )
    return out.astype(np.float32), res


def kernel(**inputs):
    out, _ = run(inputs, trace=False)
    return out
